# revision 14
# baseline (speedup 1.0000x reference)
"""Trainium2 Bass kernel for DirectionalConv2D (wind-directed 5x5 Gaussian blur).

Reference math (per pixel):
    theta = arctan2(v, u+1e-8);  c, s = cos(theta), sin(theta)
    w(dx,dy) = exp(-(dx*c + dy*s)^2 / 4.5)        for dx,dy in [-2..2]
    spread   = sum(w * fire[h+dx, w+dy]) / (sum(w) + 1e-8)   (zero padded)
    out      = clip(0.7*spread + 0.3*fire, 0, 1)

v4: three-way balance of DVE / ACT / DMA (rel-err budget 2e-2, sits ~6e-4):
  * ss = sin^2 = v^2/r2, cs = (4/3)uv/r2 via ir2 = Exp(-Ln(r2)), ir243 =
    Exp(-Ln(r2)+ln(4/3)); 12 symmetric pair weights = 12 Exp activations
    with args affine in ss/cs/m12/m1m2.  0.7/wsum is a cos(4k theta) series.
  * DVE 2x mode everywhere hot: wind path bf16, fire/weights/MAC fp16.
  * ACT does ONLY the chain (uu, vv, ln, 2 exps) + the 12 weight exps,
    dense: the series (q, t8q scale, ser) and 0.3*fire moved to cheap DVE
    TENSOR_SCALAR ops (single input stream, 2x) + one ACT Square (t8q).
  * Six late-consumed pairsums are built by the DMA engines instead of DVE:
    plain DRAM->SBUF copy of the +tap on the sync queue, then a gpsimd
    software-DGE DRAM->SBUF transfer of the -tap with accum_op=add.  Fire
    rows 0 and 5 are never loaded into SBUF (only those pairs used them).
  * uu/vv split so Square(u) starts when wu lands (wu ordered first on both
    HW queues); wind bf16 (512KB) and fire rows 1-4 (516KB) land ~13us.
  * Raw bass; monotone per-semaphore thresholds; finals + store in halves.
"""

import sys

if "/opt/trn_rl_repo" not in sys.path:
    sys.path.insert(0, "/opt/trn_rl_repo")

import numpy as np

B, H, W = 4, 512, 512
N_CORES = 8
HS = H // 2
KI = 1.0 / 4.5
C0 = 0.040093331769199714
C1 = 0.0007997721694363273
C2 = -1.6226127085146848e-06

_NC = None


def _build_nc():
    import math

    import concourse.bass as bass
    import concourse.mybir as mybir

    dt = mybir.dt
    AF = mybir.ActivationFunctionType
    OP = mybir.AluOpType
    k = KI
    f32 = dt.float32
    f16 = dt.float16
    bf16 = dt.bfloat16

    # Elide the Bass.__init__ all-engine barrier: it only orders the
    # framework const memsets (gpsimd, <1us) against their readers; our
    # first const read (an ACT bias at ~19us) is ordered by data deps, and
    # the block-exit/teardown barriers are kept.  Saves ~3us of engine
    # start-skew sync before the DMA issues.
    _orig_barrier = bass.Bass.all_engine_barrier
    bass.Bass.all_engine_barrier = lambda self, *, sem_only=False: None
    try:
        nc = bass.Bass(detect_race_conditions=False)
    finally:
        bass.Bass.all_engine_barrier = _orig_barrier

    f6_d = nc.dram_tensor("fire6", [128, 6, 516], f16, kind="ExternalInput")
    w2_d = nc.dram_tensor("w2", [128, 2048], bf16, kind="ExternalInput")  # wu|wv
    out_d = nc.dram_tensor("out", [128, 1024], f16, kind="ExternalOutput")

    def sb(name, shape, dtype=f32):
        return nc.alloc_sbuf_tensor(name, shape, dtype).ap()

    f6 = sb("f6", [128, 6, 516], f16)     # rows 0 and 5 never DMA'd (unused)
    w2 = sb("w2_t", [128, 2048], bf16)    # cols 0:1024 wu, 1024:2048 wv
    uuvv = sb("uuvv", [128, 2048], bf16)
    uv = sb("uv", [128, 1024], bf16)
    r2 = sb("r2", [128, 1024], bf16)
    lnr = sb("lnr", [128, 1024])
    ir2 = sb("ir2", [128, 1024], bf16)
    ir243 = sb("ir243", [128, 1024], bf16)
    ss = sb("ss", [128, 1024], f16)
    cs = sb("cs", [128, 1024], f16)       # holds (4/3)*sin*cos
    m12 = sb("m12", [128, 1024], f16)
    m1m2 = sb("m1m2", [128, 1024], f16)
    qy = sb("qy", [128, 1024], f16)
    q = sb("q", [128, 1024], f16)
    t8q = sb("t8q", [128, 1024], f16)
    t8c2 = sb("t8c2", [128, 1024], f16)
    ser = sb("ser", [128, 1024], f16)
    f03 = sb("f03", [128, 1024], f16)
    inv07 = sb("inv07", [128, 1024], f16)
    accv = sb("accv", [128, 1024], f16)
    prodv = sb("prodv", [128, 1024], f16)
    spf = sb("spf", [128, 1024], f16)
    sp07 = sb("sp07", [128, 1024], f16)
    opre = sb("opre", [128, 1024], f16)
    outt = sb("outt", [128, 1024], f16)
    dummy = sb("dummy_t", [128, 1])
    dummy_in = sb("dummy_in", [128, 1])

    # pairs 1-6 (DVE pairsums, fire rows 1-4 only), pairs 7-12 (DMA-built)
    pair_order = [
        (0, 1), (0, 2), (1, 0), (1, 1), (1, -1), (1, 2),
        (1, -2), (2, 0), (2, -1), (2, 1), (2, 2), (2, -2),
    ]
    dve_pairs = pair_order[:6]
    dma_pairs = pair_order[6:]
    wts = {p: sb(f"w{p[0]}_{p[1]}", [128, 1024], f16) for p in pair_order}
    pst = {p: sb(f"ps{p[0]}_{p[1]}", [128, 1024], f16) for p in pair_order}

    espec = {
        (0, 1): ("ss", -k, 0.0),
        (0, 2): ("ss", -4 * k, 0.0),
        (1, 0): ("ss", k, -k),
        (2, 0): ("ss", 4 * k, -4 * k),
        (1, 1): ("cs", -1.5 * k, -k),
        (1, -1): ("cs", 1.5 * k, -k),
        (1, 2): ("m12", -3 * k, -k),
        (1, -2): ("m1m2", -3 * k, -k),
        (2, -1): ("m12", 3 * k, -4 * k),
        (2, 1): ("m1m2", 3 * k, -4 * k),
        (2, 2): ("cs", -6 * k, -4 * k),
        (2, -2): ("cs", 6 * k, -4 * k),
    }

    def V(dx, dy, half=None):
        if half is None:
            return f6[:, 2 + dx : 4 + dx, 2 + dy : 514 + dy]
        return f6[:, 2 + dx + half, 2 + dy : 514 + dy]

    def VD(dx, dy):  # same tap window but in DRAM
        return f6_d[:, 2 + dx : 4 + dx, 2 + dy : 514 + dy]

    def flat3(ap, half=None):
        if half is None:
            return ap.rearrange("p (a b) -> p a b", a=2)
        return ap[:, half * 512 : half * 512 + 512]

    s8 = math.sqrt(8.0)
    s2_ = math.sqrt(2.0)
    bias_vals = sorted(
        {bi for _, _, bi in espec.values()} | {1e-8, -s2_, -s8 / 2, math.log(4.0 / 3.0)}
    )

    with (
        nc.semaphore("wu_s") as WU,
        nc.semaphore("wv_s") as WV,
        nc.semaphore("fc") as FC,
        nc.semaphore("f14") as F14,
        nc.semaphore("cps") as CPS,
        nc.semaphore("cpa") as CPA,
        nc.semaphore("ps") as PS,
        nc.semaphore("sqo") as SQO,
        nc.semaphore("sa") as A,
        nc.semaphore("sv") as Vs,
        nc.semaphore("sb") as SB,
    ):
        # ACT ticks: 1 dummy, 2 uu, 3 vv, 4 lnr, 5 ir2, 6 ir243,
        # 7 w01, 8 w02, 9 w10, 10 q, 11..19 rest of exps, 20 t8q
        _eorder = [p for p in pair_order]
        exp_tick = {}
        _t = 7
        for _i, _p in enumerate(_eorder):
            if _i == 3:
                _t += 1  # q occupies tick 10
            exp_tick[_p] = _t
            _t += 1
        # DVE Vs ticks: 1 r2, 2 ss, 3 cs, 4 m12, 5 m1m2, 6/7 out halves

        with nc.Block() as block:

            @block.sync
            def _(sync):
                sync.dma_start(w2[0:64, 0:1024], w2_d[0:64, 0:1024]).then_inc(WU, 16)
                sync.dma_start(w2[0:64, 1024:2048], w2_d[0:64, 1024:2048]).then_inc(WV, 16)
                sync.dma_start(f6[0:64, 1:5:3, :], f6_d[0:64, 1:5:3, :]).then_inc(F14, 16)
                # + taps of the DMA-built pairs (plain copies, HW queue)
                for p in dma_pairs:
                    sync.dma_start(flat3(pst[p]), VD(*p)).then_inc(CPS, 16)
                sync.wait_ge(Vs, 6)
                sync.dma_start(out_d[:, 0:512], outt[:, 0:512]).then_inc(SQO, 16)

            @block.gpsimd
            def _(gpsimd):
                gpsimd.dma_start(f6[:, 2:4, :], f6_d[:, 2:4, :]).then_inc(FC, 16)
                for bi_i, val in enumerate(bias_vals):
                    if (f32, val) in nc.const_aps.aps:
                        continue
                    t = nc.alloc_sbuf_tensor(f"constb{bi_i}", [128, 1], f32)
                    gpsimd.memset(t.ap(), val)
                    nc.const_aps.aps[(f32, val)] = t.ap()
                gpsimd.memset(dummy_in, 0.0).then_inc(SB, 1)
                # - taps accumulated over the copies (software-DGE compute)
                for i, p in enumerate(dma_pairs):
                    gpsimd.wait_ge(CPS, 16 * (i + 1))
                    gpsimd.dma_start(
                        flat3(pst[p]), VD(-p[0], -p[1]), accum_op=OP.add
                    ).then_inc(PS, 16)

            @block.scalar
            def _(scalar):
                a_count = [0]

                def aop(emit):
                    emit().then_inc(A, 1)
                    a_count[0] += 1

                scalar.dma_start(w2[64:128, 0:1024], w2_d[64:128, 0:1024]).then_inc(WU, 16)
                scalar.dma_start(w2[64:128, 1024:2048], w2_d[64:128, 1024:2048]).then_inc(WV, 16)
                scalar.dma_start(f6[64:128, 1:5:3, :], f6_d[64:128, 1:5:3, :]).then_inc(F14, 16)
                scalar.wait_ge(SB, 1)
                aop(lambda: scalar.activation(dummy, dummy_in, AF.Exp))               # A1
                scalar.wait_ge(WU, 32)
                aop(lambda: scalar.activation(uuvv[:, 0:1024], w2[:, 0:1024],
                                              AF.Square, bias=1e-8))                  # A2
                scalar.wait_ge(WV, 32)
                aop(lambda: scalar.activation(uuvv[:, 1024:2048], w2[:, 1024:2048],
                                              AF.Square, bias=1e-8))                  # A3
                scalar.wait_ge(Vs, 1)
                aop(lambda: scalar.activation(lnr, r2, AF.Ln))                        # A4
                scalar.wait_ge(A, 4)  # ACT pipeline RAW on lnr
                aop(lambda: scalar.activation(ir2, lnr, AF.Exp, scale=-1.0))          # A5
                aop(lambda: scalar.activation(ir243, lnr, AF.Exp, scale=-1.0,
                                              bias=math.log(4.0 / 3.0)))              # A6

                srcmap = {"ss": (ss, 2), "cs": (cs, 3), "m12": (m12, 4), "m1m2": (m1m2, 5)}
                waited = [1]
                for i, p in enumerate(pair_order):    # A7..A19 with q at A10
                    if i == 3:
                        aop(lambda: scalar.activation(q, ss, AF.Square,
                                                      bias=-s8 / 2, scale=s8))        # A10
                    srcname, sc, bi = espec[p]
                    src, need = srcmap[srcname]
                    if need > waited[0]:
                        scalar.wait_ge(Vs, need)
                        waited[0] = need
                    aop(lambda src=src, sc=sc, bi=bi, p=p:
                        scalar.activation(wts[p], src, AF.Exp, bias=bi, scale=sc))
                aop(lambda: scalar.activation(t8q, q, AF.Square, bias=-s2_, scale=s2_))  # A20
                assert a_count[0] == 20
                scalar.wait_ge(Vs, 7)
                scalar.dma_start(out_d[:, 512:1024], outt[:, 512:1024]).then_inc(SQO, 16)

            @block.vector
            def _(vector):
                vector.wait_ge(FC, 16)
                for p in [(0, 1), (0, 2)]:
                    vector.tensor_tensor(flat3(pst[p]), V(*p), V(-p[0], -p[1]), OP.add)
                vector.tensor_scalar_mul(flat3(f03), V(0, 0), 0.3)
                vector.wait_ge(WV, 32)
                vector.tensor_tensor(uv, w2[:, 0:1024], w2[:, 1024:2048], OP.mult)
                vector.wait_ge(A, 3)
                vector.tensor_tensor(r2, uuvv[:, 0:1024], uuvv[:, 1024:2048], OP.add).then_inc(Vs, 1)
                vector.wait_ge(F14, 32)
                for p in [(1, 0), (1, 1), (1, -1)]:
                    vector.tensor_tensor(flat3(pst[p]), V(*p), V(-p[0], -p[1]), OP.add)
                vector.wait_ge(A, 5)
                vector.tensor_tensor(ss, uuvv[:, 1024:2048], ir2, OP.mult).then_inc(Vs, 1)
                p12 = (1, 2)
                vector.tensor_tensor(flat3(pst[p12]), V(*p12), V(-1, -2), OP.add)
                vector.wait_ge(A, 6)
                vector.tensor_tensor(cs, uv, ir243, OP.mult).then_inc(Vs, 1)
                vector.tensor_tensor(m12, ss, cs, OP.add).then_inc(Vs, 1)
                vector.tensor_tensor(m1m2, ss, cs, OP.subtract).then_inc(Vs, 1)
                # MAC
                awaited = [6]
                ps_waited = [0]
                for i, p in enumerate(pair_order):
                    if exp_tick[p] > awaited[0]:
                        vector.wait_ge(A, exp_tick[p])
                        awaited[0] = exp_tick[p]
                    if i >= 6:
                        need = 16 * (i - 5)
                        if need > ps_waited[0]:
                            vector.wait_ge(PS, need)
                            ps_waited[0] = need
                    tgt = accv if i == 0 else prodv
                    vector.tensor_tensor(tgt, wts[p], pst[p], OP.mult)
                    if i > 0:
                        vector.tensor_tensor(accv, accv, prodv, OP.add)
                # series tail + finals
                vector.wait_ge(A, 20)
                vector.tensor_scalar_mul(t8c2, t8q, C2)
                vector.tensor_scalar(out=ser, in0=q, scalar1=C1, scalar2=C0 - C1 - C2,
                                     op0=OP.mult, op1=OP.add)
                vector.tensor_tensor(inv07, t8c2, ser, OP.add)
                for h in (0, 1):
                    hs = slice(h * 512, h * 512 + 512)
                    vector.tensor_tensor(flat3(spf, h), flat3(accv, h), V(0, 0, h), OP.add)
                    vector.tensor_tensor(sp07[:, hs], spf[:, hs], inv07[:, hs], OP.mult)
                    vector.tensor_tensor(opre[:, hs], sp07[:, hs], f03[:, hs], OP.add)
                    vector.tensor_scalar(
                        out=outt[:, hs], in0=opre[:, hs], scalar1=0.0, scalar2=1.0,
                        op0=OP.max, op1=OP.min,
                    ).then_inc(Vs, 1)   # Vs 6, 7

    return nc


def _get_nc():
    global _NC
    if _NC is None:
        _NC = _build_nc()
    return _NC


def _make_in_maps(fire_map, wind_u, wind_v):
    import ml_dtypes
    from numpy.lib.stride_tricks import sliding_window_view

    bf16 = ml_dtypes.bfloat16
    in_maps = []
    for b in range(B):
        fp = np.pad(
            np.asarray(fire_map[b, 0], np.float32), ((2, 2), (2, 2))
        ).astype(np.float16)
        for t in range(2):
            shard = fp[t * HS : t * HS + HS + 4]
            f6 = np.ascontiguousarray(
                sliding_window_view(shard, (6, 516))[::2, 0], dtype=np.float16
            )
            w2 = np.empty((128, 2048), bf16)
            w2[:, 0:1024] = np.asarray(
                wind_u[b, 0, t * HS : (t + 1) * HS], np.float32
            ).reshape(128, 1024).astype(bf16)
            w2[:, 1024:2048] = np.asarray(
                wind_v[b, 0, t * HS : (t + 1) * HS], np.float32
            ).reshape(128, 1024).astype(bf16)
            in_maps.append({"fire6": f6, "w2": w2})
    return in_maps


def _gather(results):
    out = np.empty((B, 1, H, W), np.float32)
    for ci, r in enumerate(results):
        b, t = divmod(ci, 2)
        out[b, 0, t * HS : (t + 1) * HS] = r["out"].astype(np.float32).reshape(HS, W)
    return out


def _run(fire_map, wind_u, wind_v, trace=False):
    from concourse.bass_utils import run_bass_kernel_spmd

    in_maps = _make_in_maps(fire_map, wind_u, wind_v)
    res = run_bass_kernel_spmd(_get_nc(), in_maps, list(range(N_CORES)), trace=trace)
    return _gather(res.results), res


def kernel(fire_map, wind_u, wind_v):
    out, _ = _run(fire_map, wind_u, wind_v, trace=False)
    return out


# revision 15
# speedup vs baseline: 1.0375x; 1.0375x over previous
"""Trainium2 Bass kernel for DirectionalConv2D (wind-directed 5x5 Gaussian blur).

Reference math (per pixel):
    theta = arctan2(v, u+1e-8);  c, s = cos(theta), sin(theta)
    w(dx,dy) = exp(-(dx*c + dy*s)^2 / 4.5)        for dx,dy in [-2..2]
    spread   = sum(w * fire[h+dx, w+dy]) / (sum(w) + 1e-8)   (zero padded)
    out      = clip(0.7*spread + 0.3*fire, 0, 1)

v4: three-way balance of DVE / ACT / DMA (rel-err budget 2e-2, sits ~6e-4):
  * ss = sin^2 = v^2/r2, cs = (4/3)uv/r2 via ir2 = Exp(-Ln(r2)), ir243 =
    Exp(-Ln(r2)+ln(4/3)); 12 symmetric pair weights = 12 Exp activations
    with args affine in ss/cs/m12/m1m2.  0.7/wsum is a cos(4k theta) series.
  * DVE 2x mode everywhere hot: wind path bf16, fire/weights/MAC fp16.
  * ACT does ONLY the chain (uu, vv, ln, 2 exps) + the 12 weight exps,
    dense: the series (q, t8q scale, ser) and 0.3*fire moved to cheap DVE
    TENSOR_SCALAR ops (single input stream, 2x) + one ACT Square (t8q).
  * Six late-consumed pairsums are built by the DMA engines instead of DVE:
    plain DRAM->SBUF copy of the +tap on the sync queue, then a gpsimd
    software-DGE DRAM->SBUF transfer of the -tap with accum_op=add.  Fire
    rows 0 and 5 are never loaded into SBUF (only those pairs used them).
  * uu/vv split so Square(u) starts when wu lands (wu ordered first on both
    HW queues); wind bf16 (512KB) and fire rows 1-4 (516KB) land ~13us.
  * Raw bass; monotone per-semaphore thresholds; finals + store in halves.
"""

import sys

if "/opt/trn_rl_repo" not in sys.path:
    sys.path.insert(0, "/opt/trn_rl_repo")

import numpy as np

B, H, W = 4, 512, 512
N_CORES = 8
HS = H // 2
KI = 1.0 / 4.5
C0 = 0.040093331769199714
C1 = 0.0007997721694363273
C2 = -1.6226127085146848e-06

_NC = None


def _build_nc():
    import math

    import concourse.bass as bass
    import concourse.mybir as mybir

    dt = mybir.dt
    AF = mybir.ActivationFunctionType
    OP = mybir.AluOpType
    k = KI
    f32 = dt.float32
    f16 = dt.float16
    bf16 = dt.bfloat16

    nc = bass.Bass(detect_race_conditions=False)

    f6_d = nc.dram_tensor("fire6", [128, 6, 516], f16, kind="ExternalInput")
    w2_d = nc.dram_tensor("w2", [128, 2048], bf16, kind="ExternalInput")  # wu|wv
    out_d = nc.dram_tensor("out", [128, 1024], f16, kind="ExternalOutput")

    def sb(name, shape, dtype=f32):
        return nc.alloc_sbuf_tensor(name, shape, dtype).ap()

    f6 = sb("f6", [128, 6, 516], f16)     # rows 0 and 5 never DMA'd (unused)
    w2 = sb("w2_t", [128, 2048], bf16)    # cols 0:1024 wu, 1024:2048 wv
    uuvv = sb("uuvv", [128, 2048], bf16)
    uv = sb("uv", [128, 1024], bf16)
    r2 = sb("r2", [128, 1024], bf16)
    lnr = sb("lnr", [128, 1024])
    ir2 = sb("ir2", [128, 1024], bf16)
    ir243 = sb("ir243", [128, 1024], bf16)
    ss = sb("ss", [128, 1024], f16)
    cs = sb("cs", [128, 1024], f16)       # holds (4/3)*sin*cos
    m12 = sb("m12", [128, 1024], f16)
    m1m2 = sb("m1m2", [128, 1024], f16)
    qy = sb("qy", [128, 1024], f16)
    q = sb("q", [128, 1024], f16)
    t8q = sb("t8q", [128, 1024], f16)
    t8c2 = sb("t8c2", [128, 1024], f16)
    ser = sb("ser", [128, 1024], f16)
    f03 = sb("f03", [128, 1024], f16)
    inv07 = sb("inv07", [128, 1024], f16)
    accv = sb("accv", [128, 1024], f16)
    prodv = sb("prodv", [128, 1024], f16)
    spf = sb("spf", [128, 1024], f16)
    sp07 = sb("sp07", [128, 1024], f16)
    opre = sb("opre", [128, 1024], f16)
    outt = sb("outt", [128, 1024], f16)
    dummy = sb("dummy_t", [128, 1])
    dummy_in = sb("dummy_in", [128, 1])

    # pairs 1-6 (DVE pairsums, fire rows 1-4 only), pairs 7-12 (DMA-built)
    pair_order = [
        (0, 1), (0, 2), (1, 0), (1, 1), (1, -1), (1, 2),
        (1, -2), (2, 0), (2, -1), (2, 1), (2, 2), (2, -2),
    ]
    dve_pairs = pair_order[:6]
    dma_pairs = pair_order[6:]
    wts = {p: sb(f"w{p[0]}_{p[1]}", [128, 1024], f16) for p in pair_order}
    pst = {p: sb(f"ps{p[0]}_{p[1]}", [128, 1024], f16) for p in pair_order}

    espec = {
        (0, 1): ("ss", -k, 0.0),
        (0, 2): ("ss", -4 * k, 0.0),
        (1, 0): ("ss", k, -k),
        (2, 0): ("ss", 4 * k, -4 * k),
        (1, 1): ("cs", -1.5 * k, -k),
        (1, -1): ("cs", 1.5 * k, -k),
        (1, 2): ("m12", -3 * k, -k),
        (1, -2): ("m1m2", -3 * k, -k),
        (2, -1): ("m12", 3 * k, -4 * k),
        (2, 1): ("m1m2", 3 * k, -4 * k),
        (2, 2): ("cs", -6 * k, -4 * k),
        (2, -2): ("cs", 6 * k, -4 * k),
    }

    def V(dx, dy, half=None):
        if half is None:
            return f6[:, 2 + dx : 4 + dx, 2 + dy : 514 + dy]
        return f6[:, 2 + dx + half, 2 + dy : 514 + dy]

    def VD(dx, dy):  # same tap window but in DRAM
        return f6_d[:, 2 + dx : 4 + dx, 2 + dy : 514 + dy]

    def flat3(ap, half=None):
        if half is None:
            return ap.rearrange("p (a b) -> p a b", a=2)
        return ap[:, half * 512 : half * 512 + 512]

    s8 = math.sqrt(8.0)
    s2_ = math.sqrt(2.0)
    bias_vals = sorted(
        {bi for _, _, bi in espec.values()} | {1e-8, -s2_, -s8 / 2, math.log(4.0 / 3.0)}
    )

    with (
        nc.semaphore("wu_s") as WU,
        nc.semaphore("wv_s") as WV,
        nc.semaphore("fc") as FC,
        nc.semaphore("f14") as F14,
        nc.semaphore("cps") as CPS,
        nc.semaphore("cpa") as CPA,
        nc.semaphore("ps") as PS,
        nc.semaphore("sqo") as SQO,
        nc.semaphore("sa") as A,
        nc.semaphore("sv") as Vs,
        nc.semaphore("sb") as SB,
    ):
        # ACT ticks: 1 dummy, 2 uu, 3 vv, 4 lnr, 5 ir2, 6 ir243,
        # 7 w01, 8 w02, 9 w10, 10 q, 11..19 rest of exps, 20 t8q
        _eorder = [p for p in pair_order]
        exp_tick = {}
        _t = 7
        for _i, _p in enumerate(_eorder):
            if _i == 3:
                _t += 1  # q occupies tick 10
            exp_tick[_p] = _t
            _t += 1
        # DVE Vs ticks: 1 r2, 2 ss, 3 cs, 4 m12, 5 m1m2, 6/7 out halves

        with nc.Block() as block:

            @block.sync
            def _(sync):
                sync.dma_start(w2[0:64, 0:1024], w2_d[0:64, 0:1024]).then_inc(WU, 16)
                sync.dma_start(w2[0:64, 1024:2048], w2_d[0:64, 1024:2048]).then_inc(WV, 16)
                sync.dma_start(f6[0:64, 1:5:3, :], f6_d[0:64, 1:5:3, :]).then_inc(F14, 16)
                # + taps of the DMA-built pairs (plain copies, HW queue)
                for p in dma_pairs:
                    sync.dma_start(flat3(pst[p]), VD(*p)).then_inc(CPS, 16)
                sync.wait_ge(Vs, 6)
                sync.dma_start(out_d[:, 0:512], outt[:, 0:512]).then_inc(SQO, 16)

            @block.gpsimd
            def _(gpsimd):
                gpsimd.dma_start(f6[:, 2:4, :], f6_d[:, 2:4, :]).then_inc(FC, 16)
                for bi_i, val in enumerate(bias_vals):
                    if (f32, val) in nc.const_aps.aps:
                        continue
                    t = nc.alloc_sbuf_tensor(f"constb{bi_i}", [128, 1], f32)
                    gpsimd.memset(t.ap(), val)
                    nc.const_aps.aps[(f32, val)] = t.ap()
                gpsimd.memset(dummy_in, 0.0).then_inc(SB, 1)
                # - taps accumulated over the copies (software-DGE compute)
                for i, p in enumerate(dma_pairs):
                    gpsimd.wait_ge(CPS, 16 * (i + 1))
                    gpsimd.dma_start(
                        flat3(pst[p]), VD(-p[0], -p[1]), accum_op=OP.add
                    ).then_inc(PS, 16)

            @block.scalar
            def _(scalar):
                a_count = [0]

                def aop(emit):
                    emit().then_inc(A, 1)
                    a_count[0] += 1

                scalar.dma_start(w2[64:128, 0:1024], w2_d[64:128, 0:1024]).then_inc(WU, 16)
                scalar.dma_start(w2[64:128, 1024:2048], w2_d[64:128, 1024:2048]).then_inc(WV, 16)
                scalar.dma_start(f6[64:128, 1:5:3, :], f6_d[64:128, 1:5:3, :]).then_inc(F14, 16)
                scalar.wait_ge(SB, 1)
                aop(lambda: scalar.activation(dummy, dummy_in, AF.Exp))               # A1
                scalar.wait_ge(WU, 32)
                aop(lambda: scalar.activation(uuvv[:, 0:1024], w2[:, 0:1024],
                                              AF.Square, bias=1e-8))                  # A2
                scalar.wait_ge(WV, 32)
                aop(lambda: scalar.activation(uuvv[:, 1024:2048], w2[:, 1024:2048],
                                              AF.Square, bias=1e-8))                  # A3
                scalar.wait_ge(Vs, 1)
                aop(lambda: scalar.activation(lnr, r2, AF.Ln))                        # A4
                scalar.wait_ge(A, 4)  # ACT pipeline RAW on lnr
                aop(lambda: scalar.activation(ir2, lnr, AF.Exp, scale=-1.0))          # A5
                aop(lambda: scalar.activation(ir243, lnr, AF.Exp, scale=-1.0,
                                              bias=math.log(4.0 / 3.0)))              # A6

                srcmap = {"ss": (ss, 2), "cs": (cs, 3), "m12": (m12, 4), "m1m2": (m1m2, 5)}
                waited = [1]
                for i, p in enumerate(pair_order):    # A7..A19 with q at A10
                    if i == 3:
                        aop(lambda: scalar.activation(q, ss, AF.Square,
                                                      bias=-s8 / 2, scale=s8))        # A10
                    srcname, sc, bi = espec[p]
                    src, need = srcmap[srcname]
                    if need > waited[0]:
                        scalar.wait_ge(Vs, need)
                        waited[0] = need
                    aop(lambda src=src, sc=sc, bi=bi, p=p:
                        scalar.activation(wts[p], src, AF.Exp, bias=bi, scale=sc))
                aop(lambda: scalar.activation(t8q, q, AF.Square, bias=-s2_, scale=s2_))  # A20
                assert a_count[0] == 20
                scalar.wait_ge(Vs, 7)
                scalar.dma_start(out_d[:, 512:1024], outt[:, 512:1024]).then_inc(SQO, 16)

            @block.vector
            def _(vector):
                vector.wait_ge(FC, 16)
                for p in [(0, 1), (0, 2)]:
                    vector.tensor_tensor(flat3(pst[p]), V(*p), V(-p[0], -p[1]), OP.add)
                vector.tensor_scalar_mul(flat3(f03), V(0, 0), 0.3)
                vector.wait_ge(WV, 32)
                vector.tensor_tensor(uv, w2[:, 0:1024], w2[:, 1024:2048], OP.mult)
                vector.wait_ge(A, 3)
                vector.tensor_tensor(r2, uuvv[:, 0:1024], uuvv[:, 1024:2048], OP.add).then_inc(Vs, 1)
                vector.wait_ge(F14, 32)
                for p in [(1, 0), (1, 1), (1, -1)]:
                    vector.tensor_tensor(flat3(pst[p]), V(*p), V(-p[0], -p[1]), OP.add)
                vector.wait_ge(A, 5)
                vector.tensor_tensor(ss, uuvv[:, 1024:2048], ir2, OP.mult).then_inc(Vs, 1)
                p12 = (1, 2)
                vector.tensor_tensor(flat3(pst[p12]), V(*p12), V(-1, -2), OP.add)
                vector.wait_ge(A, 6)
                vector.tensor_tensor(cs, uv, ir243, OP.mult).then_inc(Vs, 1)
                vector.tensor_tensor(m12, ss, cs, OP.add).then_inc(Vs, 1)
                vector.tensor_tensor(m1m2, ss, cs, OP.subtract).then_inc(Vs, 1)
                # MAC
                awaited = [6]
                ps_waited = [0]
                for i, p in enumerate(pair_order):
                    if exp_tick[p] > awaited[0]:
                        vector.wait_ge(A, exp_tick[p])
                        awaited[0] = exp_tick[p]
                    if i >= 6:
                        need = 16 * (i - 5)
                        if need > ps_waited[0]:
                            vector.wait_ge(PS, need)
                            ps_waited[0] = need
                    tgt = accv if i == 0 else prodv
                    vector.tensor_tensor(tgt, wts[p], pst[p], OP.mult)
                    if i > 0:
                        vector.tensor_tensor(accv, accv, prodv, OP.add)
                # series tail + finals
                vector.wait_ge(A, 20)
                vector.tensor_scalar_mul(t8c2, t8q, C2)
                vector.tensor_scalar(out=ser, in0=q, scalar1=C1, scalar2=C0 - C1 - C2,
                                     op0=OP.mult, op1=OP.add)
                vector.tensor_tensor(inv07, t8c2, ser, OP.add)
                for h in (0, 1):
                    hs = slice(h * 512, h * 512 + 512)
                    vector.tensor_tensor(flat3(spf, h), flat3(accv, h), V(0, 0, h), OP.add)
                    vector.tensor_tensor(sp07[:, hs], spf[:, hs], inv07[:, hs], OP.mult)
                    vector.tensor_tensor(opre[:, hs], sp07[:, hs], f03[:, hs], OP.add)
                    vector.tensor_scalar(
                        out=outt[:, hs], in0=opre[:, hs], scalar1=0.0, scalar2=1.0,
                        op0=OP.max, op1=OP.min,
                    ).then_inc(Vs, 1)   # Vs 6, 7

    return nc


def _get_nc():
    global _NC
    if _NC is None:
        _NC = _build_nc()
    return _NC


def _make_in_maps(fire_map, wind_u, wind_v):
    import ml_dtypes
    from numpy.lib.stride_tricks import sliding_window_view

    bf16 = ml_dtypes.bfloat16
    in_maps = []
    for b in range(B):
        fp = np.pad(
            np.asarray(fire_map[b, 0], np.float32), ((2, 2), (2, 2))
        ).astype(np.float16)
        for t in range(2):
            shard = fp[t * HS : t * HS + HS + 4]
            f6 = np.ascontiguousarray(
                sliding_window_view(shard, (6, 516))[::2, 0], dtype=np.float16
            )
            w2 = np.empty((128, 2048), bf16)
            w2[:, 0:1024] = np.asarray(
                wind_u[b, 0, t * HS : (t + 1) * HS], np.float32
            ).reshape(128, 1024).astype(bf16)
            w2[:, 1024:2048] = np.asarray(
                wind_v[b, 0, t * HS : (t + 1) * HS], np.float32
            ).reshape(128, 1024).astype(bf16)
            in_maps.append({"fire6": f6, "w2": w2})
    return in_maps


def _gather(results):
    out = np.empty((B, 1, H, W), np.float32)
    for ci, r in enumerate(results):
        b, t = divmod(ci, 2)
        out[b, 0, t * HS : (t + 1) * HS] = r["out"].astype(np.float32).reshape(HS, W)
    return out


def _run(fire_map, wind_u, wind_v, trace=False):
    from concourse.bass_utils import run_bass_kernel_spmd

    in_maps = _make_in_maps(fire_map, wind_u, wind_v)
    res = run_bass_kernel_spmd(_get_nc(), in_maps, list(range(N_CORES)), trace=trace)
    return _gather(res.results), res


def kernel(fire_map, wind_u, wind_v):
    out, _ = _run(fire_map, wind_u, wind_v, trace=False)
    return out


# revision 16
# speedup vs baseline: 1.0405x; 1.0029x over previous
"""Trainium2 Bass kernel for DirectionalConv2D (wind-directed 5x5 Gaussian blur).

Reference math (per pixel):
    theta = arctan2(v, u+1e-8);  c, s = cos(theta), sin(theta)
    w(dx,dy) = exp(-(dx*c + dy*s)^2 / 4.5)        for dx,dy in [-2..2]
    spread   = sum(w * fire[h+dx, w+dy]) / (sum(w) + 1e-8)   (zero padded)
    out      = clip(0.7*spread + 0.3*fire, 0, 1)

v4: three-way balance of DVE / ACT / DMA (rel-err budget 2e-2, sits ~6e-4):
  * ss = sin^2 = v^2/r2, cs = (4/3)uv/r2 via ir2 = Exp(-Ln(r2)), ir243 =
    Exp(-Ln(r2)+ln(4/3)); 12 symmetric pair weights = 12 Exp activations
    with args affine in ss/cs/m12/m1m2.  0.7/wsum is a cos(4k theta) series.
  * DVE 2x mode everywhere hot: wind path bf16, fire/weights/MAC fp16.
  * ACT does ONLY the chain (uu, vv, ln, 2 exps) + the 12 weight exps,
    dense: the series (q, t8q scale, ser) and 0.3*fire moved to cheap DVE
    TENSOR_SCALAR ops (single input stream, 2x) + one ACT Square (t8q).
  * Six late-consumed pairsums are built by the DMA engines instead of DVE:
    plain DRAM->SBUF copy of the +tap on the sync queue, then a gpsimd
    software-DGE DRAM->SBUF transfer of the -tap with accum_op=add.  Fire
    rows 0 and 5 are never loaded into SBUF (only those pairs used them).
  * uu/vv split so Square(u) starts when wu lands (wu ordered first on both
    HW queues); wind bf16 (512KB) and fire rows 1-4 (516KB) land ~13us.
  * Raw bass; monotone per-semaphore thresholds; finals + store in halves.
"""

import sys

if "/opt/trn_rl_repo" not in sys.path:
    sys.path.insert(0, "/opt/trn_rl_repo")

import numpy as np

B, H, W = 4, 512, 512
N_CORES = 8
HS = H // 2
KI = 1.0 / 4.5
C0 = 0.040093331769199714
C1 = 0.0007997721694363273
C2 = -1.6226127085146848e-06

_NC = None


def _build_nc():
    import math

    import concourse.bass as bass
    import concourse.mybir as mybir

    dt = mybir.dt
    AF = mybir.ActivationFunctionType
    OP = mybir.AluOpType
    k = KI
    f32 = dt.float32
    f16 = dt.float16
    bf16 = dt.bfloat16

    nc = bass.Bass(detect_race_conditions=False)

    f6_d = nc.dram_tensor("fire6", [128, 6, 516], f16, kind="ExternalInput")
    w2_d = nc.dram_tensor("w2", [128, 2048], bf16, kind="ExternalInput")  # wu|wv
    out_d = nc.dram_tensor("out", [128, 1024], f16, kind="ExternalOutput")

    def sb(name, shape, dtype=f32):
        return nc.alloc_sbuf_tensor(name, shape, dtype).ap()

    f6 = sb("f6", [128, 6, 516], f16)     # rows 0 and 5 never DMA'd (unused)
    w2 = sb("w2_t", [128, 2048], bf16)    # cols 0:1024 wu, 1024:2048 wv
    uuvv = sb("uuvv", [128, 2048], bf16)
    uv = sb("uv", [128, 1024], bf16)
    r2 = sb("r2", [128, 1024], bf16)
    lnr = sb("lnr", [128, 1024])
    ir2 = sb("ir2", [128, 1024], bf16)
    ir243 = sb("ir243", [128, 1024], bf16)
    ss = sb("ss", [128, 1024], f16)
    cs = sb("cs", [128, 1024], f16)       # holds (4/3)*sin*cos
    m12 = sb("m12", [128, 1024], f16)
    m1m2 = sb("m1m2", [128, 1024], f16)
    qy = sb("qy", [128, 1024], f16)
    q = sb("q", [128, 1024], f16)
    t8q = sb("t8q", [128, 1024], f16)
    t8c2 = sb("t8c2", [128, 1024], f16)
    ser = sb("ser", [128, 1024], f16)
    f03 = sb("f03", [128, 1024], f16)
    inv07 = sb("inv07", [128, 1024], f16)
    accv = sb("accv", [128, 1024], f16)
    prodv = sb("prodv", [128, 1024], f16)
    spf = sb("spf", [128, 1024], f16)
    sp07 = sb("sp07", [128, 1024], f16)
    opre = sb("opre", [128, 1024], f16)
    outt = sb("outt", [128, 1024], f16)
    dummy = sb("dummy_t", [128, 1])
    dummy_in = sb("dummy_in", [128, 1])

    # pairs 1-6 (DVE pairsums, fire rows 1-4 only), pairs 7-12 (DMA-built)
    pair_order = [
        (0, 1), (0, 2), (1, 0), (1, 1), (1, -1), (1, 2),
        (1, -2), (2, 0), (2, -1), (2, 1), (2, 2), (2, -2),
    ]
    dve_pairs = [(0, 1), (0, 2), (1, 0), (1, 1), (1, 2)]
    dma_pairs = [(1, -1)] + pair_order[6:]
    wts = {p: sb(f"w{p[0]}_{p[1]}", [128, 1024], f16) for p in pair_order}
    pst = {p: sb(f"ps{p[0]}_{p[1]}", [128, 1024], f16) for p in pair_order}

    espec = {
        (0, 1): ("ss", -k, 0.0),
        (0, 2): ("ss", -4 * k, 0.0),
        (1, 0): ("ss", k, -k),
        (2, 0): ("ss", 4 * k, -4 * k),
        (1, 1): ("cs", -1.5 * k, -k),
        (1, -1): ("cs", 1.5 * k, -k),
        (1, 2): ("m12", -3 * k, -k),
        (1, -2): ("m1m2", -3 * k, -k),
        (2, -1): ("m12", 3 * k, -4 * k),
        (2, 1): ("m1m2", 3 * k, -4 * k),
        (2, 2): ("cs", -6 * k, -4 * k),
        (2, -2): ("cs", 6 * k, -4 * k),
    }

    def V(dx, dy, half=None):
        if half is None:
            return f6[:, 2 + dx : 4 + dx, 2 + dy : 514 + dy]
        return f6[:, 2 + dx + half, 2 + dy : 514 + dy]

    def VD(dx, dy):  # same tap window but in DRAM
        return f6_d[:, 2 + dx : 4 + dx, 2 + dy : 514 + dy]

    def flat3(ap, half=None):
        if half is None:
            return ap.rearrange("p (a b) -> p a b", a=2)
        return ap[:, half * 512 : half * 512 + 512]

    s8 = math.sqrt(8.0)
    s2_ = math.sqrt(2.0)
    bias_vals = sorted(
        {bi for _, _, bi in espec.values()} | {1e-8, -s2_, -s8 / 2, math.log(4.0 / 3.0)}
    )

    with (
        nc.semaphore("wu_s") as WU,
        nc.semaphore("wv_s") as WV,
        nc.semaphore("fc") as FC,
        nc.semaphore("f14") as F14,
        nc.semaphore("cps") as CPS,
        nc.semaphore("cpa") as CPA,
        nc.semaphore("ps") as PS,
        nc.semaphore("sqo") as SQO,
        nc.semaphore("sa") as A,
        nc.semaphore("sv") as Vs,
        nc.semaphore("sb") as SB,
    ):
        # ACT ticks: 1 dummy, 2 uu, 3 vv, 4 lnr, 5 ir2, 6 ir243,
        # 7 w01, 8 w02, 9 w10, 10 q, 11..19 rest of exps, 20 t8q
        _eorder = [p for p in pair_order]
        exp_tick = {}
        _t = 7
        for _i, _p in enumerate(_eorder):
            if _i == 3:
                _t += 1  # q occupies tick 10
            exp_tick[_p] = _t
            _t += 1
        # DVE Vs ticks: 1 r2, 2 ss, 3 cs, 4 m12, 5 m1m2, 6/7 out halves

        with nc.Block() as block:

            @block.sync
            def _(sync):
                sync.dma_start(w2[0:64, 0:1024], w2_d[0:64, 0:1024]).then_inc(WU, 16)
                sync.dma_start(w2[0:64, 1024:2048], w2_d[0:64, 1024:2048]).then_inc(WV, 16)
                sync.dma_start(f6[0:64, 1:5:3, :], f6_d[0:64, 1:5:3, :]).then_inc(F14, 16)
                # + taps of the DMA-built pairs (plain copies, HW queue)
                for p in dma_pairs:
                    sync.dma_start(flat3(pst[p]), VD(*p)).then_inc(CPS, 16)
                sync.wait_ge(Vs, 6)
                sync.dma_start(out_d[:, 0:512], outt[:, 0:512]).then_inc(SQO, 16)

            @block.gpsimd
            def _(gpsimd):
                gpsimd.dma_start(f6[:, 2:4, :], f6_d[:, 2:4, :]).then_inc(FC, 16)
                for bi_i, val in enumerate(bias_vals):
                    if (f32, val) in nc.const_aps.aps:
                        continue
                    t = nc.alloc_sbuf_tensor(f"constb{bi_i}", [128, 1], f32)
                    gpsimd.memset(t.ap(), val)
                    nc.const_aps.aps[(f32, val)] = t.ap()
                gpsimd.memset(dummy_in, 0.0).then_inc(SB, 1)
                # - taps accumulated over the copies (software-DGE compute)
                for i, p in enumerate(dma_pairs):
                    gpsimd.wait_ge(CPS, 16 * (i + 1))
                    gpsimd.dma_start(
                        flat3(pst[p]), VD(-p[0], -p[1]), accum_op=OP.add
                    ).then_inc(PS, 16)

            @block.scalar
            def _(scalar):
                a_count = [0]

                def aop(emit):
                    emit().then_inc(A, 1)
                    a_count[0] += 1

                scalar.dma_start(w2[64:128, 0:1024], w2_d[64:128, 0:1024]).then_inc(WU, 16)
                scalar.dma_start(w2[64:128, 1024:2048], w2_d[64:128, 1024:2048]).then_inc(WV, 16)
                scalar.dma_start(f6[64:128, 1:5:3, :], f6_d[64:128, 1:5:3, :]).then_inc(F14, 16)
                scalar.wait_ge(SB, 1)
                aop(lambda: scalar.activation(dummy, dummy_in, AF.Exp))               # A1
                scalar.wait_ge(WU, 32)
                aop(lambda: scalar.activation(uuvv[:, 0:1024], w2[:, 0:1024],
                                              AF.Square, bias=1e-8))                  # A2
                scalar.wait_ge(WV, 32)
                aop(lambda: scalar.activation(uuvv[:, 1024:2048], w2[:, 1024:2048],
                                              AF.Square, bias=1e-8))                  # A3
                scalar.wait_ge(Vs, 1)
                aop(lambda: scalar.activation(lnr, r2, AF.Ln))                        # A4
                scalar.wait_ge(A, 4)  # ACT pipeline RAW on lnr
                aop(lambda: scalar.activation(ir2, lnr, AF.Exp, scale=-1.0))          # A5
                aop(lambda: scalar.activation(ir243, lnr, AF.Exp, scale=-1.0,
                                              bias=math.log(4.0 / 3.0)))              # A6

                srcmap = {"ss": (ss, 2), "cs": (cs, 3), "m12": (m12, 4), "m1m2": (m1m2, 5)}
                waited = [1]
                for i, p in enumerate(pair_order):    # A7..A19 with q at A10
                    if i == 3:
                        aop(lambda: scalar.activation(q, ss, AF.Square,
                                                      bias=-s8 / 2, scale=s8))        # A10
                    srcname, sc, bi = espec[p]
                    src, need = srcmap[srcname]
                    if need > waited[0]:
                        scalar.wait_ge(Vs, need)
                        waited[0] = need
                    aop(lambda src=src, sc=sc, bi=bi, p=p:
                        scalar.activation(wts[p], src, AF.Exp, bias=bi, scale=sc))
                aop(lambda: scalar.activation(t8q, q, AF.Square, bias=-s2_, scale=s2_))  # A20
                assert a_count[0] == 20
                scalar.wait_ge(Vs, 7)
                scalar.dma_start(out_d[:, 512:1024], outt[:, 512:1024]).then_inc(SQO, 16)

            @block.vector
            def _(vector):
                vector.wait_ge(FC, 16)
                for p in [(0, 1), (0, 2)]:
                    vector.tensor_tensor(flat3(pst[p]), V(*p), V(-p[0], -p[1]), OP.add)
                vector.tensor_scalar_mul(flat3(f03), V(0, 0), 0.3)
                vector.wait_ge(WV, 32)
                vector.tensor_tensor(uv, w2[:, 0:1024], w2[:, 1024:2048], OP.mult)
                vector.wait_ge(A, 3)
                vector.tensor_tensor(r2, uuvv[:, 0:1024], uuvv[:, 1024:2048], OP.add).then_inc(Vs, 1)
                vector.wait_ge(F14, 32)
                for p in [(1, 0), (1, 1)]:
                    vector.tensor_tensor(flat3(pst[p]), V(*p), V(-p[0], -p[1]), OP.add)
                vector.wait_ge(A, 5)
                vector.tensor_tensor(ss, uuvv[:, 1024:2048], ir2, OP.mult).then_inc(Vs, 1)
                p12 = (1, 2)
                vector.tensor_tensor(flat3(pst[p12]), V(*p12), V(-1, -2), OP.add)
                vector.wait_ge(A, 6)
                vector.tensor_tensor(cs, uv, ir243, OP.mult).then_inc(Vs, 1)
                vector.tensor_tensor(m12, ss, cs, OP.add).then_inc(Vs, 1)
                vector.tensor_tensor(m1m2, ss, cs, OP.subtract).then_inc(Vs, 1)
                # MAC
                awaited = [6]
                ps_waited = [0]
                for i, p in enumerate(pair_order):
                    if exp_tick[p] > awaited[0]:
                        vector.wait_ge(A, exp_tick[p])
                        awaited[0] = exp_tick[p]
                    if i == 4 or i >= 6:
                        need = 16 if i == 4 else 16 * (i - 4)
                        if need > ps_waited[0]:
                            vector.wait_ge(PS, need)
                            ps_waited[0] = need
                    tgt = accv if i == 0 else prodv
                    vector.tensor_tensor(tgt, wts[p], pst[p], OP.mult)
                    if i > 0:
                        vector.tensor_tensor(accv, accv, prodv, OP.add)
                # series tail + finals
                vector.wait_ge(A, 20)
                vector.tensor_scalar_mul(t8c2, t8q, C2)
                vector.tensor_scalar(out=ser, in0=q, scalar1=C1, scalar2=C0 - C1 - C2,
                                     op0=OP.mult, op1=OP.add)
                vector.tensor_tensor(inv07, t8c2, ser, OP.add)
                for h in (0, 1):
                    hs = slice(h * 512, h * 512 + 512)
                    vector.tensor_tensor(flat3(spf, h), flat3(accv, h), V(0, 0, h), OP.add)
                    vector.tensor_tensor(sp07[:, hs], spf[:, hs], inv07[:, hs], OP.mult)
                    vector.tensor_tensor(opre[:, hs], sp07[:, hs], f03[:, hs], OP.add)
                    vector.tensor_scalar(
                        out=outt[:, hs], in0=opre[:, hs], scalar1=0.0, scalar2=1.0,
                        op0=OP.max, op1=OP.min,
                    ).then_inc(Vs, 1)   # Vs 6, 7

    return nc


def _get_nc():
    global _NC
    if _NC is None:
        _NC = _build_nc()
    return _NC


def _make_in_maps(fire_map, wind_u, wind_v):
    import ml_dtypes
    from numpy.lib.stride_tricks import sliding_window_view

    bf16 = ml_dtypes.bfloat16
    in_maps = []
    for b in range(B):
        fp = np.pad(
            np.asarray(fire_map[b, 0], np.float32), ((2, 2), (2, 2))
        ).astype(np.float16)
        for t in range(2):
            shard = fp[t * HS : t * HS + HS + 4]
            f6 = np.ascontiguousarray(
                sliding_window_view(shard, (6, 516))[::2, 0], dtype=np.float16
            )
            w2 = np.empty((128, 2048), bf16)
            w2[:, 0:1024] = np.asarray(
                wind_u[b, 0, t * HS : (t + 1) * HS], np.float32
            ).reshape(128, 1024).astype(bf16)
            w2[:, 1024:2048] = np.asarray(
                wind_v[b, 0, t * HS : (t + 1) * HS], np.float32
            ).reshape(128, 1024).astype(bf16)
            in_maps.append({"fire6": f6, "w2": w2})
    return in_maps


def _gather(results):
    out = np.empty((B, 1, H, W), np.float32)
    for ci, r in enumerate(results):
        b, t = divmod(ci, 2)
        out[b, 0, t * HS : (t + 1) * HS] = r["out"].astype(np.float32).reshape(HS, W)
    return out


def _run(fire_map, wind_u, wind_v, trace=False):
    from concourse.bass_utils import run_bass_kernel_spmd

    in_maps = _make_in_maps(fire_map, wind_u, wind_v)
    res = run_bass_kernel_spmd(_get_nc(), in_maps, list(range(N_CORES)), trace=trace)
    return _gather(res.results), res


def kernel(fire_map, wind_u, wind_v):
    out, _ = _run(fire_map, wind_u, wind_v, trace=False)
    return out
